# revision 1
# baseline (speedup 1.0000x reference)
"""Binary CNN (dense_cnn) Trainium2 kernel — 8-core pure data parallel.

Network (per reference): 4 binarized convs + BN/hardtanh (+2 maxpools) + FC.
All sign()-nonlinearities are folded into per-channel threshold compares on
the raw conv accumulators (BN scale > 0 makes sign(affine(x)) a threshold op),
so the device pipeline is: conv -> (pool) -> threshold -> next conv, with the
continuous path (BN4 affine + hardtanh + FC) only at the end.

Layouts: channels on SBUF partitions, (n, h, w) in the free dim. conv1 is done
as a K=10 (9 taps + zero row) matmul against a tap-skewed replica of sign(x)
built via a DRAM staging round-trip (even/odd w split so the stride-2 conv
becomes stride-1 gathers); 16 concurrent PE tiles (4 row-bases x 4 col-slices).
conv2/3 contract channels with the 3 w-taps as sequentially accumulated
matmuls over shifted free-dim views; conv4 contracts its 6 h-taps the same
way. The FC runs activation-stationary (lhsT = h4) so the output lands with
samples on partitions, making the final DMA coarse.
"""

import numpy as np
import ml_dtypes

import concourse.bass as bass
import concourse.bacc as bacc
import concourse.tile as tile
import concourse.mybir as mybir

F32 = mybir.dt.float32
BF16 = mybir.dt.bfloat16
F8 = mybir.dt.float8e4
NPF8 = ml_dtypes.float8_e4m3
BN_EPS = 1e-5

N_CORES = 8
N_TOTAL = 8192
N_CORE = N_TOTAL // N_CORES  # 1024
B = 128  # samples per chunk
ALU = mybir.AluOpType
ACTF = mybir.ActivationFunctionType


# ---------------------------------------------------------------------------
# host-side parameter preparation (pure numpy)
# ---------------------------------------------------------------------------
def host_prep(p):
    def s(k):
        return p[f"g{k}"] / np.sqrt(p[f"v{k}"] + BN_EPS)

    w1b = np.sign(p["w1"]).astype(np.float32)  # (32,1,1,9)
    w2b = np.sign(p["w2"]).astype(np.float32)  # (64,32,1,3)
    w3b = np.sign(p["w3"]).astype(np.float32)  # (128,64,1,3)
    w4b = np.sign(p["w4"]).astype(np.float32)  # (128,128,6,1)
    s1, s2, s3, s4 = s(1), s(2), s(3), s(4)
    thr1 = (p["m1"] - p["b1"] - p["be1"] / s1).astype(np.float32)  # (32,)
    thr2 = (p["m2"] - p["b2"] - p["be2"] / s2).astype(np.float32)  # (64,)
    S3 = w3b.sum(axis=(1, 2, 3)).astype(np.float32)
    thr3 = ((S3 - p["b3"] + p["m3"] - p["be3"] / s3) / 2).astype(np.float32)
    scale4 = s4.astype(np.float32)
    bias4 = ((p["b4"] - p["m4"]) * s4 + p["be4"]).astype(np.float32)

    # conv1 lhsT row order: even taps {0,2,4,6,8} then odd taps {1,3,5,7},
    # matching the two contiguous-partition skew DMAs; row 9 stays zero.
    w1l = np.zeros((128, 32), NPF8)
    tap_order = [0, 2, 4, 6, 8, 1, 3, 5, 7]
    for r in range(4):
        w1l[32 * r : 32 * r + 9, :] = (
            w1b[:, 0, 0, tap_order].T.astype(NPF8)
        )
    w2l = np.zeros((128, 192), NPF8)
    for r in range(4):
        for t in range(3):
            w2l[32 * r : 32 * r + 32, t * 64 : (t + 1) * 64] = (
                w2b[:, :, 0, t].T.astype(NPF8)
            )
    w3l = np.zeros((128, 384), NPF8)
    for r in range(2):
        for t in range(3):
            w3l[64 * r : 64 * r + 64, t * 128 : (t + 1) * 128] = (
                w3b[:, :, 0, t].T.astype(NPF8)
            )
    w4l = np.zeros((128, 768), NPF8)
    for h in range(6):
        w4l[:, h * 128 : (h + 1) * 128] = w4b[:, :, h, 0].T.astype(NPF8)
    wfcl = np.zeros((128, 160), np.float32)
    wfc = p["wfc"].astype(np.float32)  # (10, 2048), idx = c*16+w
    for w in range(16):
        wfcl[:, w * 10 : (w + 1) * 10] = wfc[:, w::16].T  # [c, j]

    return {
        "w1l": w1l,
        "w2l": w2l,
        "w3l": w3l,
        "w4l": w4l,
        "wfcl": wfcl,
        "thr1n": np.tile(-thr1, 4).reshape(128, 1).astype(np.float32),
        "thr2t": np.tile(thr2, 2).reshape(128, 1).astype(np.float32),
        "thr3n": (-thr3).reshape(128, 1).astype(np.float32),
        "sc4t": scale4.reshape(128, 1),
        "bi4t": bias4.reshape(128, 1),
        "bfct": np.tile(p["bfc"].astype(np.float32), (128, 1)),  # (128,10)
    }


PARAM_SPECS = [
    ("w1l", [128, 32], F8),
    ("w2l", [128, 192], F8),
    ("w3l", [128, 384], F8),
    ("w4l", [128, 768], F8),
    ("wfcl", [128, 160], F32),
    ("thr1n", [128, 1], F32),
    ("thr2t", [128, 1], F32),
    ("thr3n", [128, 1], F32),
    ("sc4t", [128, 1], F32),
    ("bi4t", [128, 1], F32),
    ("bfct", [128, 10], F32),
]


# ---------------------------------------------------------------------------
# device program
# ---------------------------------------------------------------------------
def build_program(n_core=N_CORE, num_devices=N_CORES):
    nc = bacc.Bacc("TRN2", num_devices=num_devices)
    x = nc.dram_tensor("x", [n_core, 6, 128], F32, kind="ExternalInput").ap()
    params = {
        name: nc.dram_tensor(name, shape, dt, kind="ExternalInput").ap()
        for name, shape, dt in PARAM_SPECS
    }
    out = nc.dram_tensor("out", [n_core, 10], F32, kind="ExternalOutput").ap()
    xeo_d = nc.dram_tensor("xeo_scratch", [n_core, 6, 2, 72], F8).ap()

    with tile.TileContext(nc) as tc:
        _emit(nc, tc, x, params, out, xeo_d, n_core)
    nc.compile()
    return nc


def _emit(nc, tc, x, P, out, xeo_d, n_core):
    from contextlib import ExitStack

    ctx = ExitStack()
    chunks = n_core // B
    singles = ctx.enter_context(tc.tile_pool(name="singles", bufs=1))
    big = ctx.enter_context(tc.tile_pool(name="big", bufs=1))
    small = ctx.enter_context(tc.tile_pool(name="small", bufs=4))
    x9p = ctx.enter_context(tc.tile_pool(name="x9p", bufs=4))
    psum = ctx.enter_context(tc.tile_pool(name="psum", bufs=8, space="PSUM"))

    # constants
    sb = {}
    for name, shape, dt in PARAM_SPECS:
        sb[name] = singles.tile(shape, dt, name=f"{name}_sb")
        nc.gpsimd.dma_start(out=sb[name], in_=P[name])

    for ci in range(chunks):
        n0c = ci * B
        # ---- stage A: load x chunk, binarize into even/odd staging --------
        xin = big.tile([128, 6, 128], F32, tag="xin")
        nc.gpsimd.dma_start(out=xin, in_=x[n0c : n0c + B])
        xeo = big.tile([128, 6, 2, 72], F8, tag="xeo")
        nc.vector.memset(xeo[:, :, :, 0:2], 0.0)
        nc.vector.memset(xeo[:, :, :, 66:72], 0.0)
        nc.scalar.activation(xeo[:, :, 0, 2:66], xin[:, :, 0:128:2], ACTF.Sign)
        nc.scalar.activation(xeo[:, :, 1, 2:66], xin[:, :, 1:128:2], ACTF.Sign)
        nc.gpsimd.dma_start(out=xeo_d[n0c : n0c + B], in_=xeo)

        # ---- stage B: conv1 (16-tile) -> Sign (ACT) -> pool (TT max) ------
        # h1pre: per-position sign bits (+-1 fp8) for the whole chunk;
        # pooling happens on SBUF afterwards (TT cannot read two PSUM views)
        h1pre = big.tile([128, 8, 4, 6, 64], F8, tag="h1pre")
        for rnd in range(8):
            x9 = x9p.tile([128, 24, 64], F8, tag="x9")
            for r in range(4):
                n0 = n0c + rnd * 16 + r * 4
                for par in range(2):  # even taps -> partitions 32r+0..5,
                    src = bass.AP(  # odd taps -> partitions 32r+5..10
                        tensor=xeo_d.tensor,
                        offset=n0 * 864 + 72 * par,
                        ap=[[1, 5], [144, 24], [1, 64]],
                    )
                    dst = x9[32 * r + 5 * par : 32 * r + 5 * par + 5]
                    nc.sync.dma_start(out=dst, in_=src)
            pp1 = [
                psum.tile([128, 384], F32, tag="pp", name=f"pp1_{rnd}_{r}")
                for r in range(4)
            ]
            for r in range(4):
                for c in range(4):
                    nc.tensor.matmul(
                        pp1[r][32 * c : 32 * c + 32],
                        lhsT=sb["w1l"][32 * r : 32 * r + 10],
                        rhs=x9[32 * r : 32 * r + 10, 6 * c : 6 * c + 6, :],
                        start=True,
                        stop=True,
                        tile_position=(32 * r, 32 * c),
                    )
            for r in range(4):
                nc.scalar.activation(
                    h1pre[:, rnd, r],
                    pp1[r].rearrange("p (h w) -> p h w", h=6),
                    ACTF.Sign,
                    bias=sb["thr1n"],
                )
        # pool pairs along w; sign(max) == max(sign). h1b holds the 4
        # n-classes (n mod 4 == c) at partition base 32c so conv2 can run
        # 4 concurrent row-tiles.
        h1b = big.tile([128, 32, 6, 34], F8, tag="h1b")
        nc.vector.memset(h1b[:, :, :, 0:1], 0.0)
        nc.vector.memset(h1b[:, :, :, 33:34], 0.0)
        for c in range(4):
            pslice = slice(32 * c, 32 * c + 32)
            nc.vector.tensor_tensor(
                h1b[pslice, :, :, 1:33],
                h1pre[pslice, :, :, :, 0:64:2].rearrange(
                    "p a b h w -> p (a b) h w"
                ),
                h1pre[pslice, :, :, :, 1:64:2].rearrange(
                    "p a b h w -> p (a b) h w"
                ),
                ALU.max,
            )

        # ---- stage C: conv2 (4 row-tiles x 2 col-slots) -> q2 in {0,1} ----
        # q2 layout: partition half = sample-subgroup, f slot = 8j+2c+i for
        # sample n = 16j + 4t + c (t = 2m+i); conv3 reads L/H halves as two
        # concurrent row-tiles over the same f slots.
        q2 = big.tile([128, 64, 6, 34], F8, tag="q2")
        nc.vector.memset(q2[:, :, :, 0:1], 0.5)
        nc.vector.memset(q2[:, :, :, 33:34], 0.5)
        for j in range(8):
            pp2 = [
                psum.tile([128, 384], F32, tag="pp", name=f"pp2_{j}_{c}")
                for c in range(4)
            ]
            for m in range(2):  # col slot (sequential acc groups per bank)
                for t in range(3):
                    for c in range(4):  # row-tiles, concurrent
                        k0 = 4 * j + 2 * m
                        nc.tensor.matmul(
                            pp2[c][64 * m : 64 * m + 64],
                            lhsT=sb["w2l"][
                                32 * c : 32 * c + 32, t * 64 : (t + 1) * 64
                            ],
                            rhs=h1b[
                                32 * c : 32 * c + 32, k0 : k0 + 2, :, t : t + 32
                            ],
                            start=(t == 0),
                            stop=(t == 2),
                            tile_position=(32 * c, 64 * m),
                        )
            for c in range(4):
                nc.vector.tensor_scalar(
                    q2[:, 8 * j + 2 * c : 8 * j + 2 * c + 2, :, 1:33],
                    pp2[c].rearrange("p (n h w) -> p n h w", n=2, h=6),
                    sb["thr2t"],
                    None,
                    ALU.is_ge,
                )

        # ---- stage D: conv3 (2 row-tiles) -> Sign -> pool -> h3b ----------
        h3pre = big.tile([128, 128, 6, 32], F8, tag="h3pre")
        for rnd in range(32):  # 4 samples per round via L/H row-tiles
            j, c = rnd // 4, rnd % 4
            s0 = 8 * j + 2 * c
            pp3 = [
                psum.tile([128, 384], F32, tag="pp", name=f"pp3_{rnd}_{g}")
                for g in range(2)
            ]
            for t in range(3):
                for g in range(2):  # row-tile halves, concurrent
                    nc.tensor.matmul(
                        pp3[g],
                        lhsT=sb["w3l"][
                            64 * g : 64 * g + 64, t * 128 : (t + 1) * 128
                        ],
                        rhs=q2[
                            64 * g : 64 * g + 64, s0 : s0 + 2, :, t : t + 32
                        ],
                        start=(t == 0),
                        stop=(t == 2),
                        tile_position=(64 * g, 0),
                    )
            for g in range(2):
                # samples {16j+c+8g, 16j+c+8g+4} -> strided n slice
                na = 16 * j + c + 8 * g
                nc.scalar.activation(
                    h3pre[:, na : na + 5 : 4],
                    pp3[g].rearrange("p (n h w) -> p n h w", n=2, h=6),
                    ACTF.Sign,
                    bias=sb["thr3n"],
                )
        h3b = big.tile([128, 128, 6, 16], F8, tag="h3b")
        for g in range(2):
            nc.vector.tensor_tensor(
                h3b[:, 64 * g : 64 * g + 64],
                h3pre[:, 64 * g : 64 * g + 64, :, 0:32:2],
                h3pre[:, 64 * g : 64 * g + 64, :, 1:32:2],
                ALU.max,
            )

        # ---- stage E: conv4 + BN4 + hardtanh -> h4 (fp32) -----------------
        h4 = big.tile([128, 128, 16], F32, tag="h4")
        for rnd in range(4):
            pp4 = psum.tile([128, 512], F32, tag="pp")
            for hh in range(6):
                nc.tensor.matmul(
                    pp4,
                    lhsT=sb["w4l"][:, hh * 128 : (hh + 1) * 128],
                    rhs=h3b[:, 32 * rnd : 32 * rnd + 32, hh, :],
                    start=(hh == 0),
                    stop=(hh == 5),
                )
            t4 = small.tile([128, 512], F32, tag="t4")
            nc.vector.tensor_scalar(
                t4, pp4, sb["sc4t"], sb["bi4t"], ALU.mult, ALU.add
            )
            nc.vector.tensor_scalar(
                h4[:, 32 * rnd : 32 * rnd + 32].rearrange("p n w -> p (n w)"),
                t4,
                1.0,
                -1.0,
                ALU.min,
                ALU.max,
            )

        # ---- stage F: FC (activation-stationary) + bias -------------------
        ppf = psum.tile([128, 16], F32, tag="pp")
        for w in range(16):
            nc.tensor.matmul(
                ppf[:, 0:10],
                lhsT=h4[:, :, w : w + 1],
                rhs=sb["wfcl"][:, w * 10 : (w + 1) * 10],
                start=(w == 0),
                stop=(w == 15),
            )
        osb = small.tile([128, 10], F32, tag="osb")
        nc.vector.tensor_tensor(osb, ppf[:, 0:10], sb["bfct"], ALU.add)
        nc.sync.dma_start(out=out[n0c : n0c + B], in_=osb)
    ctx.close()


# ---------------------------------------------------------------------------
# entry point
# ---------------------------------------------------------------------------
_PROGRAM = None


def kernel(**inputs):
    from concourse.bass_utils import run_bass_kernel_spmd

    global _PROGRAM
    if _PROGRAM is None:
        _PROGRAM = build_program()
    nc = _PROGRAM

    derived = host_prep(inputs)
    x = np.ascontiguousarray(inputs["x"], dtype=np.float32).reshape(
        N_TOTAL, 6, 128
    )
    in_maps = []
    for i in range(N_CORES):
        m = dict(derived)
        m["x"] = x[i * N_CORE : (i + 1) * N_CORE]
        in_maps.append(m)
    res = run_bass_kernel_spmd(nc, in_maps, core_ids=list(range(N_CORES)))
    return np.concatenate(
        [res.results[i]["out"] for i in range(N_CORES)], axis=0
    ).astype(np.float32)



# revision 2
# speedup vs baseline: 2849.3042x; 2849.3042x over previous
"""Binary CNN Trainium2 kernel — 8-core data parallel, DoubleRow-centric.

Host packs sign(x) into stride-2 tap-pair rows so conv1 is one DoubleRow
matmul per 4-sample group (K=40 block-diagonal, M=128) with zero on-device
input prep and no DRAM staging. conv2/conv3 use a flat (6h x 34w + pad)
halo'd slot layout ([slot, 256] fp8) so each tile is exactly 2 stride-2
DoubleRow matmuls (taps (0,2) then (1,zero)) at FD=512; conv4 is 3 DoubleRow
matmuls over h-pairs. Class pairs / channel halves sit at different
partition bases so their matmuls share the PE array on real HW.

Elementwise work is balanced across ACT and DVE: conv1's boundary fuses the
odd-w threshold with the pool-max via scalar_tensor_tensor (DVE) after an
ACT Sign on the even half; conv2's threshold runs on ACT (+-1 q2); conv3's
threshold splits ACT (Sign, +-1) / DVE (is_ge*2, {0,2}) by sample range with
a ones-row psum correction in conv4, then a DVE tensor_tensor max pools into
h3b. BN4+hardtanh+FC fold into a per-channel clamp plus host-folded FC
weights; FC is weight-stationary over 4-chunk blocks producing [10, N] that
the host transposes.
"""

import numpy as np
import ml_dtypes

import concourse.bass as bass
import concourse.bacc as bacc
import concourse.tile as tile
import concourse.mybir as mybir

F32 = mybir.dt.float32
BF16 = mybir.dt.bfloat16
F8 = mybir.dt.float8e4
NPF8 = ml_dtypes.float8_e4m3
NPBF16 = ml_dtypes.bfloat16
BN_EPS = 1e-5

N_CORES = 8
N_TOTAL = 8192
N_CORE = N_TOTAL // N_CORES  # 1024
B = 128  # samples per chunk
ALU = mybir.AluOpType
ACTF = mybir.ActivationFunctionType
DR = mybir.MatmulPerfMode.DoubleRow
T16_SPLIT = 10  # conv3 tiles t16 < split: ACT (+-1); rest: DVE ({0,2})

# conv1 tap-pair rows: (base shift, parity); sub0 -> shift base, sub1 -> base+2
ROWS1 = [(-2, 0), (-1, 0), (2, 0), (-2, 1), (-1, 1)]
# taps covered: (r, sub) -> original tap index t (None = zero weight)
TAP1 = [(0, 4), (2, 6), (8, None), (1, 5), (3, 7)]


def subap(sl, dims, extra=0):
    return bass.AP(
        tensor=sl.tensor, offset=sl.offset + extra, ap=[list(sl.ap[0])] + dims
    )


# ---------------------------------------------------------------------------
# host-side preparation
# ---------------------------------------------------------------------------
def host_prep_params(p):
    def s(k):
        return (p[f"g{k}"] / np.sqrt(p[f"v{k}"] + BN_EPS)).astype(np.float32)

    w1b = np.sign(p["w1"]).astype(np.float32)[:, 0, 0, :]  # (32, 9)
    w2b = np.sign(p["w2"]).astype(np.float32)[:, :, 0, :]  # (64, 32, 3)
    w3b = np.sign(p["w3"]).astype(np.float32)[:, :, 0, :]  # (128, 64, 3)
    w4b = np.sign(p["w4"]).astype(np.float32)[:, :, :, 0]  # (128, 128, 6)
    s1, s2, s3, s4 = s(1), s(2), s(3), s(4)
    thr1 = (p["m1"] - p["b1"] - p["be1"] / s1).astype(np.float32)  # (32,)
    thr2 = (p["m2"] - p["b2"] - p["be2"] / s2).astype(np.float32)  # (64,)
    S2 = w2b.sum(axis=(1, 2))
    thr2_01 = ((thr2 + S2) / 2).astype(np.float32)  # h1b {0,1} coded
    thr3 = (p["m3"] - p["be3"] / s3 - p["b3"]).astype(np.float32)  # q2 +-1 coded
    S4 = w4b.sum(axis=(1, 2)).astype(np.float32)  # {0,2} coding correction

    # conv1 lhsT [20, 2, 128]
    w1dr = np.zeros((20, 2, 128), NPF8)
    for si in range(4):
        for r in range(5):
            for k in range(2):
                t = TAP1[r][k]
                if t is not None:
                    w1dr[5 * si + r, k, 32 * si : 32 * si + 32] = w1b[:, t].astype(
                        NPF8
                    )

    # conv2 lhsT [128, 2, 128]: rows (32*ci2 + c), cols (64*ci2 + d); dup halves
    w2A = np.zeros((128, 2, 128), NPF8)
    w2B = np.zeros((128, 2, 128), NPF8)
    for ci2 in range(2):
        w2A[32 * ci2 : 32 * ci2 + 32, 0, 64 * ci2 : 64 * ci2 + 64] = w2b[
            :, :, 0
        ].T.astype(NPF8)
        w2A[32 * ci2 : 32 * ci2 + 32, 1, 64 * ci2 : 64 * ci2 + 64] = w2b[
            :, :, 2
        ].T.astype(NPF8)
        w2B[32 * ci2 : 32 * ci2 + 32, 0, 64 * ci2 : 64 * ci2 + 64] = w2b[
            :, :, 1
        ].T.astype(NPF8)
    w2A[64:128] = w2A[0:64]
    w2B[64:128] = w2B[0:64]

    # conv3 lhsT [128, 2, 128]: rows c (64), cols e (128); dup halves
    w3A = np.zeros((128, 2, 128), NPF8)
    w3B = np.zeros((128, 2, 128), NPF8)
    w3A[0:64, 0, :] = w3b[:, :, 0].T.astype(NPF8)
    w3A[0:64, 1, :] = w3b[:, :, 2].T.astype(NPF8)
    w3B[0:64, 0, :] = w3b[:, :, 1].T.astype(NPF8)
    w3A[64:128] = w3A[0:64]
    w3B[64:128] = w3B[0:64]

    # conv4 DoubleRow lhsT over h-pairs: 3 tiles [128, 2, 128]: [c, k, o]
    w4A = np.ascontiguousarray(w4b[:, :, 0:2].transpose(1, 2, 0)).astype(NPF8)
    w4B = np.ascontiguousarray(w4b[:, :, 2:4].transpose(1, 2, 0)).astype(NPF8)
    w4C = np.ascontiguousarray(w4b[:, :, 4:6].transpose(1, 2, 0)).astype(NPF8)

    # BN4 + hardtanh + FC folds (h3b +-1 coded; {0,2} ranges get a -S4
    # ones-row inside psum so one clamp/bias set serves all samples).
    b4 = p["b4"].astype(np.float32)
    m4 = p["m4"].astype(np.float32)
    be4 = p["be4"].astype(np.float32)
    A4 = (-b4 + m4 - (1.0 + be4) / s4).astype(np.float32)
    B4 = (-b4 + m4 + (1.0 - be4) / s4).astype(np.float32)
    D4 = (s4 * (b4 - m4) + be4).astype(np.float32)
    wfc = p["wfc"].astype(np.float32)  # (10, 2048), k = c*16 + w
    wfc2 = wfc * np.repeat(s4, 16)[None, :]
    bfc2 = (p["bfc"].astype(np.float32) + wfc @ np.repeat(D4, 16)).astype(
        np.float32
    )
    wfcl = np.zeros((128, 16, 10), NPBF16)
    for w in range(16):
        wfcl[:, w, :] = wfc2[:, w::16].T.astype(NPBF16)  # [c, j]
    bias2d = np.ascontiguousarray(
        np.broadcast_to(bfc2[:, None], (10, 512))
    ).astype(np.float32)

    def col(v):
        return np.ascontiguousarray(v.reshape(-1, 1)).astype(np.float32)

    return {
        "w1dr": w1dr,
        "w2A": w2A,
        "w2B": w2B,
        "w3A": w3A,
        "w3B": w3B,
        "w4A": w4A,
        "w4B": w4B,
        "w4C": w4C,
        "w4c": np.ascontiguousarray((-S4).reshape(1, 128)).astype(NPBF16),
        "msk4": np.ascontiguousarray(
            np.repeat((np.arange(32) >= 8).astype(np.float32), 16).reshape(1, 512)
        ).astype(NPBF16),
        "wfcl": wfcl,
        "bias2d": bias2d,
        "thr1n": col(np.tile(-thr1, 4)),
        "thr1t": col(np.tile(thr1, 4)),
        "thr2n": col(np.tile(-thr2_01, 2)),
        "thr3n": col(-thr3),
        "thr3t": col(thr3),
        "clB": col(B4),
        "clA": col(A4),
    }


_NIDX = None  # per-chunk sample permutation n(s, g) = 64*(s%2) + 32*(s//2) + g


def _nidx():
    global _NIDX
    if _NIDX is None:
        idx = np.zeros((4, 32), np.int64)
        for si in range(4):
            idx[si] = 64 * (si % 2) + 32 * (si // 2) + np.arange(32)
        _NIDX = idx
    return _NIDX


def host_pack_x(x):
    """x (N, 6, 128) f32 -> x9 (N//128, 20, 32*6*68) f8."""
    N = x.shape[0]
    xb = np.sign(x).astype(NPF8)
    par = xb.reshape(N, 6, 64, 2)  # [..., w2, parity]
    rows = np.zeros((N, 5, 6, 68), NPF8)
    for r, (base, parity) in enumerate(ROWS1):
        src = par[..., parity]  # (N, 6, 64)
        lo = max(0, -base)
        hi = min(66, 64 - base)
        rows[:, r, :, lo:hi] = src[:, :, base + lo : base + hi]
    chunks = N // B
    rows = rows.reshape(chunks, B, 5, 6 * 68)
    x9 = np.empty((chunks, 4, 32, 5, 6 * 68), NPF8)
    idx = _nidx()
    for si in range(4):
        x9[:, si] = rows[:, idx[si]]
    x9 = np.ascontiguousarray(x9.transpose(0, 1, 3, 2, 4))
    return x9.reshape(chunks, 20, 32 * 6 * 68)


PARAM_SPECS = [
    ("w1dr", [20, 2, 128], F8),
    ("w2A", [128, 2, 128], F8),
    ("w2B", [128, 2, 128], F8),
    ("w3A", [128, 2, 128], F8),
    ("w3B", [128, 2, 128], F8),
    ("w4A", [128, 2, 128], F8),
    ("w4B", [128, 2, 128], F8),
    ("w4C", [128, 2, 128], F8),
    ("w4c", [1, 128], BF16),
    ("msk4", [1, 512], BF16),
    ("wfcl", [128, 16, 10], BF16),
    ("bias2d", [10, 512], F32),
    ("thr1n", [128, 1], F32),
    ("thr1t", [128, 1], F32),
    ("thr2n", [128, 1], F32),
    ("thr3n", [128, 1], F32),
    ("thr3t", [128, 1], F32),
    ("clB", [128, 1], F32),
    ("clA", [128, 1], F32),
]


# ---------------------------------------------------------------------------
# device program
# ---------------------------------------------------------------------------
def build_program(n_core=N_CORE, num_devices=N_CORES, debug_dump=False):
    nc = bacc.Bacc("TRN2", num_devices=num_devices)
    chunks = n_core // B
    x9_d = nc.dram_tensor(
        "x9", [chunks, 20, 32 * 6 * 68], F8, kind="ExternalInput"
    ).ap()
    P = {
        name: nc.dram_tensor(name, shape, dt, kind="ExternalInput").ap()
        for name, shape, dt in PARAM_SPECS
    }
    out = nc.dram_tensor("out", [10, n_core], F32, kind="ExternalOutput").ap()
    dump = None
    if debug_dump:
        dump = {
            "h3dump": nc.dram_tensor(
                "h3dump", [128, 128, 6, 16], F32, kind="ExternalOutput"
            ).ap(),
            "h4dump": nc.dram_tensor(
                "h4dump", [128, 128, 16], F32, kind="ExternalOutput"
            ).ap(),
        }
    with tile.TileContext(nc) as tc:
        _emit(nc, tc, x9_d, P, out, chunks, dump=dump)
    nc.compile()
    return nc


def _emit(nc, tc, x9_d, P, out, chunks, dump=None):
    from contextlib import ExitStack

    ctx = ExitStack()
    FB = min(4, chunks)  # chunks per FC block
    singles = ctx.enter_context(tc.tile_pool(name="singles", bufs=1))
    psA = ctx.enter_context(tc.tile_pool(name="psA", bufs=3, space="PSUM"))
    psB = ctx.enter_context(tc.tile_pool(name="psB", bufs=2, space="PSUM"))
    tmp3p = ctx.enter_context(tc.tile_pool(name="tmp3p", bufs=4))
    osbp = ctx.enter_context(tc.tile_pool(name="osbp", bufs=2))

    sb = {}
    for name, shape, dt in PARAM_SPECS:
        sb[name] = singles.tile(shape, dt, name=f"{name}_sb")
        nc.sync.dma_start(out=sb[name], in_=P[name])

    # persistent double-buffers
    x9b = [singles.tile([20, 32, 6 * 68], F8, name=f"x9_{i}") for i in range(2)]
    tmp1b = [singles.tile([128, 32, 6, 32], F8, name=f"tmp1_{i}") for i in range(2)]
    h1bb = [singles.tile([128, 34, 256], F8, name=f"h1b_{i}") for i in range(2)]
    q2b = [singles.tile([128, 66, 256], F8, name=f"q2_{i}") for i in range(2)]
    h3bb = [singles.tile([128, 128, 6, 16], F8, name=f"h3b_{i}") for i in range(2)]
    h4b = [
        singles.tile([128, FB * 128, 16], BF16, name=f"h4_{i}") for i in range(2)
    ]
    for i in range(2):
        nc.vector.memset(h1bb[i], 0.5)  # {0,1} coding: pad/halo = 0.5
        nc.gpsimd.memset(q2b[i], 0.0)  # +-1 coding: pad/halo = 0

    def emit_dma(ci):
        nc.sync.dma_start(out=x9b[ci % 2], in_=x9_d[ci])

    def conv1_tile(ci, gp):
        """2 DR matmuls; ACT signs even-w into tmp1, DVE STT fuses the odd-w
        threshold with the pool-max into h1b ({0,1})."""
        x9t, tmp1 = x9b[ci % 2], tmp1b[ci % 2]
        h1bt = h1bb[ci % 2]
        pp = psA.tile([128, 2, 512], F32, tag="pp")
        for k in range(2):
            g = 2 * gp + k
            rhs = subap(x9t[:, g, 0:1], [[2, 2], [68, 6], [1, 64]])
            nc.tensor.matmul(
                pp[:, k, 0:384],
                lhsT=sb["w1dr"],
                rhs=rhs,
                start=True,
                stop=True,
                perf_mode=DR,
                tile_position=(0, 0),
            )
        ppv = pp[:, :, 0:384].rearrange("p a (h w) -> p a h w", h=6)
        te = tmp1[:, 2 * gp : 2 * gp + 2]
        nc.scalar.activation(te, ppv[:, :, :, 0:64:2], ACTF.Sign, bias=sb["thr1n"])
        for k in range(2):
            o = subap(
                h1bt[:, 0, 0:1],
                [[34, 6], [1, 32]],
                extra=(1 + 2 * gp + k) * 256 + 1,
            )
            nc.vector.scalar_tensor_tensor(
                o, ppv[:, k, :, 1:64:2], sb["thr1t"], te[:, k], ALU.is_ge, ALU.max
            )

    def conv2_tile(ci, k16):
        """2 slot-pairs (one 2-bank psum tile) + ACT Sign -> q2 (+-1)."""
        Pcp, jp = k16 // 8, k16 % 8
        h1bt, q2t = h1bb[ci % 2], q2b[ci % 2]
        pp = psA.tile([128, 2, 512], F32, tag="pp")
        for b2 in range(2):
            u0 = 1 + 4 * jp + 2 * b2
            for st, wname, d in ((True, "w2A", -1), (False, "w2B", 0)):
                rhs = subap(
                    h1bt[64 * Pcp : 64 * Pcp + 64, 0, 0:1],
                    [[2, 2], [1, 512]],
                    extra=u0 * 256 + d,
                )
                nc.tensor.matmul(
                    pp[:, b2, :],
                    lhsT=sb[wname][64 * Pcp : 64 * Pcp + 64],
                    rhs=rhs,
                    start=st,
                    stop=not st,
                    perf_mode=DR,
                    tile_position=(64 * Pcp, 0),
                )
        inp = subap(pp[:, 0, 0:1], [[512, 2], [256, 2], [34, 6], [1, 32]], extra=1)
        o = subap(
            q2t[:, 0, 0:1],
            [[512, 2], [256, 2], [34, 6], [1, 32]],
            extra=(1 + 32 * Pcp + 4 * jp) * 256 + 1,
        )
        nc.scalar.activation(o, inp, ACTF.Sign, bias=sb["thr2n"])

    def conv3_group(ci, gq, ta, tb):
        """2 psum tiles (t16 = ta, tb), thresholds, one pool op -> h3b.
        t16 < T16_SPLIT: ACT Sign (+-1); else DVE is_ge*2 ({0,2})."""
        q2t, h3bt = q2b[ci % 2], h3bb[ci % 2]
        t3 = tmp3p.tile([128, 8, 192], F8, tag="t3")
        for ti, t16 in enumerate((ta, tb)):
            pp = psA.tile([128, 2, 512], F32, tag="pp")
            for b2 in range(2):
                u0 = 1 + 4 * t16 + 2 * b2
                for st, wname, d in ((True, "w3A", -1), (False, "w3B", 0)):
                    rhs = subap(
                        q2t[64 * gq : 64 * gq + 64, 0, 0:1],
                        [[2, 2], [1, 512]],
                        extra=u0 * 256 + d,
                    )
                    nc.tensor.matmul(
                        pp[:, b2, :],
                        lhsT=sb[wname][64 * gq : 64 * gq + 64],
                        rhs=rhs,
                        start=st,
                        stop=not st,
                        perf_mode=DR,
                        tile_position=(64 * gq, 0),
                    )
            inp = subap(
                pp[:, 0, 0:1], [[512, 2], [256, 2], [34, 6], [1, 32]], extra=1
            )
            o = t3[:, 4 * ti : 4 * ti + 4, :].rearrange(
                "p s (h w) -> p s h w", h=6
            )
            if t16 < T16_SPLIT:
                nc.scalar.activation(o, inp, ACTF.Sign, bias=sb["thr3n"])
            else:
                nc.vector.tensor_scalar(
                    o, inp, sb["thr3t"], 2.0, ALU.is_ge, ALU.mult
                )
        sig0 = 64 * gq + 4 * ta
        o = subap(
            h3bt[:, 0, 0, 0:1],
            [[(tb - ta) * 384, 2], [96, 4], [16, 6], [1, 16]],
            extra=sig0 * 96,
        )
        in0 = subap(t3[:, 0, 0:1], [[768, 2], [192, 4], [32, 6], [2, 16]], extra=0)
        in1 = subap(t3[:, 0, 0:1], [[768, 2], [192, 4], [32, 6], [2, 16]], extra=1)
        nc.vector.tensor_tensor(o, in0, in1, ALU.max)

    # conv4 rounds r4 cover h3b slots [32*r4, 32*r4+32); within rounds 1 and
    # 3 the slots past 4*T16_SPLIT-32 are {0,2}-coded -> masked -S4 ones-row.

    def conv4_round(ci, r4):
        h3bt, h4t = h3bb[ci % 2], h4b[(ci // FB) % 2]
        pp4 = psB.tile([128, 512], F32, tag="pp4")
        ones = r4 % 2 == 1  # rounds 1 and 3 contain the {0,2}-coded slots
        for hp, wname in enumerate(("w4A", "w4B", "w4C")):
            rhs = subap(
                h3bt[:, 0, 0, 0:1],
                [[16, 2], [96, 32], [1, 16]],
                extra=32 * r4 * 96 + 2 * hp * 16,
            )
            nc.tensor.matmul(
                pp4,
                lhsT=sb[wname],
                rhs=rhs,
                start=(hp == 0),
                stop=(hp == 2 and not ones),
                perf_mode=DR,
                tile_position=(0, 0),
            )
        if ones:
            nc.tensor.matmul(
                pp4,
                lhsT=sb["w4c"],
                rhs=sb["msk4"],
                start=False,
                stop=True,
                tile_position=(0, 0),
            )
        nc.vector.tensor_scalar(
            h4t[
                :, 128 * (ci % FB) + 32 * r4 : 128 * (ci % FB) + 32 * r4 + 32, :
            ].rearrange("p n w -> p (n w)"),
            pp4,
            sb["clB"],
            sb["clA"],
            ALU.min,
            ALU.max,
        )

    def emit_fc(ci):
        ND = FB * 128
        h4t = h4b[(ci // FB) % 2]
        ppf = psB.tile([128, 512], F32, tag="pp4")
        for w in range(16):
            nc.tensor.matmul(
                ppf[0:10, 0:ND],
                lhsT=sb["wfcl"][:, w, :],
                rhs=h4t[:, :, w],
                start=(w == 0),
                stop=(w == 15),
            )
        osb = osbp.tile([16, 512], F32, tag="osb")
        nc.vector.tensor_tensor(
            osb[0:10, 0:ND], ppf[0:10, 0:ND], sb["bias2d"][:, 0:ND], ALU.add
        )
        blk = ci // FB
        nc.sync.dma_start(
            out=out[:, blk * ND : (blk + 1) * ND], in_=osb[0:10, 0:ND]
        )

    # ---- emission: software-pipelined across chunks -----------------------
    # conv3 groups pair an ACT-coded tile with a DVE-coded one where possible
    PAIRS3 = [(0, 10), (1, 11), (2, 12), (3, 13), (4, 14), (5, 15), (6, 7), (8, 9)]
    ORD3 = [(gq, ta, tb) for ta, tb in PAIRS3 for gq in range(2)]
    emit_dma(0)
    for gp in range(16):
        conv1_tile(0, gp)
    if chunks > 1:
        emit_dma(1)
    for k16 in range(16):
        conv2_tile(0, k16)
        if chunks > 1:
            conv1_tile(1, k16)
    for ci in range(chunks):
        if ci + 2 < chunks:
            emit_dma(ci + 2)
        for k16 in range(16):
            gq, ta, tb = ORD3[k16]
            conv3_group(ci, gq, ta, tb)
            if ci + 1 < chunks:
                conv2_tile(ci + 1, k16)
            if ci + 2 < chunks:
                conv1_tile(ci + 2, k16)
        for r4 in range(4):
            conv4_round(ci, r4)
        if ci % FB == FB - 1:
            emit_fc(ci)
    if dump is not None:
        h3f = singles.tile([128, 128, 6, 16], F32, name="h3f_dbg")
        nc.vector.tensor_copy(h3f, h3bb[0])
        nc.sync.dma_start(out=dump["h3dump"], in_=h3f)
        h4f = singles.tile([128, 128, 16], F32, name="h4f_dbg")
        nc.vector.tensor_copy(h4f, h4b[0][:, 0:128, :])
        nc.sync.dma_start(out=dump["h4dump"], in_=h4f)
    ctx.close()


# ---------------------------------------------------------------------------
# entry point
# ---------------------------------------------------------------------------
_PROGRAM = None


def kernel(**inputs):
    from concourse.bass_utils import run_bass_kernel_spmd

    global _PROGRAM
    if _PROGRAM is None:
        _PROGRAM = build_program()
    nc = _PROGRAM

    params = host_prep_params(inputs)
    x = np.ascontiguousarray(inputs["x"], dtype=np.float32).reshape(N_TOTAL, 6, 128)
    x9 = host_pack_x(x)  # (64, 20, 13056)
    chunks = N_CORE // B
    in_maps = []
    for i in range(N_CORES):
        m = dict(params)
        m["x9"] = x9[i * chunks : (i + 1) * chunks]
        in_maps.append(m)
    res = run_bass_kernel_spmd(nc, in_maps, core_ids=list(range(N_CORES)))
    outs = [res.results[i]["out"].T for i in range(N_CORES)]  # (1024, 10) each
    return np.ascontiguousarray(np.concatenate(outs, axis=0)).astype(np.float32)


# revision 3
# speedup vs baseline: 2863.9401x; 1.0051x over previous
"""Binary CNN Trainium2 kernel — 8-core data parallel, DoubleRow-centric.

Host packs sign(x) into stride-2 tap-pair rows so conv1 is one DoubleRow
matmul per 4-sample group (K=40 block-diagonal, M=128) with zero on-device
input prep and no DRAM staging. conv2/conv3 use a flat (6h x 34w + pad)
halo'd slot layout ([slot, 256] fp8) so each tile is exactly 2 stride-2
DoubleRow matmuls (taps (0,2) then (1,zero)) at FD=512; conv4 is 3 DoubleRow
matmuls over h-pairs. Class pairs / channel halves sit at different
partition bases so their matmuls share the PE array on real HW.

Elementwise work is balanced across ACT and DVE: conv1's boundary fuses the
odd-w threshold with the pool-max via scalar_tensor_tensor (DVE) after an
ACT Sign on the even half; conv2's threshold runs on ACT (+-1 q2); conv3's
threshold splits ACT (Sign, +-1) / DVE (is_ge*2, {0,2}) by sample range with
a ones-row psum correction in conv4, then a DVE tensor_tensor max pools into
h3b. BN4+hardtanh+FC fold into a per-channel clamp plus host-folded FC
weights; FC is weight-stationary over 4-chunk blocks producing [10, N] that
the host transposes.
"""

import numpy as np
import ml_dtypes

import concourse.bass as bass
import concourse.bacc as bacc
import concourse.tile as tile
import concourse.mybir as mybir

F32 = mybir.dt.float32
BF16 = mybir.dt.bfloat16
F8 = mybir.dt.float8e4
NPF8 = ml_dtypes.float8_e4m3
NPBF16 = ml_dtypes.bfloat16
BN_EPS = 1e-5

N_CORES = 8
N_TOTAL = 8192
N_CORE = N_TOTAL // N_CORES  # 1024
B = 128  # samples per chunk
ALU = mybir.AluOpType
ACTF = mybir.ActivationFunctionType
DR = mybir.MatmulPerfMode.DoubleRow
T16_SPLIT = 10  # conv3 tiles t16 < split: ACT (+-1); rest: DVE ({0,2})

# conv1 tap-pair rows: (base shift, parity); sub0 -> shift base, sub1 -> base+2
ROWS1 = [(-2, 0), (-1, 0), (2, 0), (-2, 1), (-1, 1)]
# taps covered: (r, sub) -> original tap index t (None = zero weight)
TAP1 = [(0, 4), (2, 6), (8, None), (1, 5), (3, 7)]


def subap(sl, dims, extra=0):
    return bass.AP(
        tensor=sl.tensor, offset=sl.offset + extra, ap=[list(sl.ap[0])] + dims
    )


# ---------------------------------------------------------------------------
# host-side preparation
# ---------------------------------------------------------------------------
def host_prep_params(p):
    def s(k):
        return (p[f"g{k}"] / np.sqrt(p[f"v{k}"] + BN_EPS)).astype(np.float32)

    w1b = np.sign(p["w1"]).astype(np.float32)[:, 0, 0, :]  # (32, 9)
    w2b = np.sign(p["w2"]).astype(np.float32)[:, :, 0, :]  # (64, 32, 3)
    w3b = np.sign(p["w3"]).astype(np.float32)[:, :, 0, :]  # (128, 64, 3)
    w4b = np.sign(p["w4"]).astype(np.float32)[:, :, :, 0]  # (128, 128, 6)
    s1, s2, s3, s4 = s(1), s(2), s(3), s(4)
    thr1 = (p["m1"] - p["b1"] - p["be1"] / s1).astype(np.float32)  # (32,)
    thr2 = (p["m2"] - p["b2"] - p["be2"] / s2).astype(np.float32)  # (64,)
    S2 = w2b.sum(axis=(1, 2))
    thr2_01 = ((thr2 + S2) / 2).astype(np.float32)  # h1b {0,1} coded
    thr3 = (p["m3"] - p["be3"] / s3 - p["b3"]).astype(np.float32)  # q2 +-1 coded
    S4 = w4b.sum(axis=(1, 2)).astype(np.float32)  # {0,2} coding correction

    # conv1 lhsT [20, 2, 128]
    w1dr = np.zeros((20, 2, 128), NPF8)
    for si in range(4):
        for r in range(5):
            for k in range(2):
                t = TAP1[r][k]
                if t is not None:
                    w1dr[5 * si + r, k, 32 * si : 32 * si + 32] = w1b[:, t].astype(
                        NPF8
                    )

    # conv2 lhsT [128, 2, 128]: rows (32*ci2 + c), cols (64*ci2 + d); dup halves
    w2A = np.zeros((128, 2, 128), NPF8)
    w2B = np.zeros((128, 2, 128), NPF8)
    for ci2 in range(2):
        w2A[32 * ci2 : 32 * ci2 + 32, 0, 64 * ci2 : 64 * ci2 + 64] = w2b[
            :, :, 0
        ].T.astype(NPF8)
        w2A[32 * ci2 : 32 * ci2 + 32, 1, 64 * ci2 : 64 * ci2 + 64] = w2b[
            :, :, 2
        ].T.astype(NPF8)
        w2B[32 * ci2 : 32 * ci2 + 32, 0, 64 * ci2 : 64 * ci2 + 64] = w2b[
            :, :, 1
        ].T.astype(NPF8)
    w2A[64:128] = w2A[0:64]
    w2B[64:128] = w2B[0:64]

    # conv3 lhsT [128, 2, 128]: rows c (64), cols e (128); dup halves
    w3A = np.zeros((128, 2, 128), NPF8)
    w3B = np.zeros((128, 2, 128), NPF8)
    w3A[0:64, 0, :] = w3b[:, :, 0].T.astype(NPF8)
    w3A[0:64, 1, :] = w3b[:, :, 2].T.astype(NPF8)
    w3B[0:64, 0, :] = w3b[:, :, 1].T.astype(NPF8)
    w3A[64:128] = w3A[0:64]
    w3B[64:128] = w3B[0:64]

    # conv4 DoubleRow lhsT over h-pairs: 3 tiles [128, 2, 128]: [c, k, o]
    w4A = np.ascontiguousarray(w4b[:, :, 0:2].transpose(1, 2, 0)).astype(NPF8)
    w4B = np.ascontiguousarray(w4b[:, :, 2:4].transpose(1, 2, 0)).astype(NPF8)
    w4C = np.ascontiguousarray(w4b[:, :, 4:6].transpose(1, 2, 0)).astype(NPF8)

    # BN4 + hardtanh + FC folds (h3b +-1 coded; {0,2} ranges get a -S4
    # ones-row inside psum so one clamp/bias set serves all samples).
    b4 = p["b4"].astype(np.float32)
    m4 = p["m4"].astype(np.float32)
    be4 = p["be4"].astype(np.float32)
    A4 = (-b4 + m4 - (1.0 + be4) / s4).astype(np.float32)
    B4 = (-b4 + m4 + (1.0 - be4) / s4).astype(np.float32)
    D4 = (s4 * (b4 - m4) + be4).astype(np.float32)
    wfc = p["wfc"].astype(np.float32)  # (10, 2048), k = c*16 + w
    wfc2 = wfc * np.repeat(s4, 16)[None, :]
    bfc2 = (p["bfc"].astype(np.float32) + wfc @ np.repeat(D4, 16)).astype(
        np.float32
    )
    wfcl = np.zeros((128, 16, 10), NPBF16)
    for w in range(16):
        wfcl[:, w, :] = wfc2[:, w::16].T.astype(NPBF16)  # [c, j]
    bias2d = np.ascontiguousarray(
        np.broadcast_to(bfc2[:, None], (10, 512))
    ).astype(np.float32)

    def col(v):
        return np.ascontiguousarray(v.reshape(-1, 1)).astype(np.float32)

    return {
        "w1dr": w1dr,
        "w2A": w2A,
        "w2B": w2B,
        "w3A": w3A,
        "w3B": w3B,
        "w4A": w4A,
        "w4B": w4B,
        "w4C": w4C,
        "w4c": np.ascontiguousarray((-S4).reshape(1, 128)).astype(NPBF16),
        "msk4": np.ascontiguousarray(
            np.repeat((np.arange(32) >= 8).astype(np.float32), 16).reshape(1, 512)
        ).astype(NPBF16),
        "wfcl": wfcl,
        "bias2d": bias2d,
        "thr1n": col(np.tile(-thr1, 4)),
        "thr1t": col(np.tile(thr1, 4)),
        "thr2n": col(np.tile(-thr2_01, 2)),
        "thr3n": col(-thr3),
        "thr3t": col(thr3),
        "clB": col(B4),
        "clA": col(A4),
    }


_NIDX = None  # per-chunk sample permutation n(s, g) = 64*(s%2) + 32*(s//2) + g


def _nidx():
    global _NIDX
    if _NIDX is None:
        idx = np.zeros((4, 32), np.int64)
        for si in range(4):
            idx[si] = 64 * (si % 2) + 32 * (si // 2) + np.arange(32)
        _NIDX = idx
    return _NIDX


def host_pack_x(x):
    """x (N, 6, 128) f32 -> x9 (N//128, 20, 32*6*68) f8."""
    N = x.shape[0]
    xb = np.sign(x).astype(NPF8)
    par = xb.reshape(N, 6, 64, 2)  # [..., w2, parity]
    rows = np.zeros((N, 5, 6, 68), NPF8)
    for r, (base, parity) in enumerate(ROWS1):
        src = par[..., parity]  # (N, 6, 64)
        lo = max(0, -base)
        hi = min(66, 64 - base)
        rows[:, r, :, lo:hi] = src[:, :, base + lo : base + hi]
    chunks = N // B
    rows = rows.reshape(chunks, B, 5, 6 * 68)
    x9 = np.empty((chunks, 4, 32, 5, 6 * 68), NPF8)
    idx = _nidx()
    for si in range(4):
        x9[:, si] = rows[:, idx[si]]
    x9 = np.ascontiguousarray(x9.transpose(0, 1, 3, 2, 4))
    return x9.reshape(chunks, 20, 32 * 6 * 68)


PARAM_SPECS = [
    ("w1dr", [20, 2, 128], F8),
    ("w2A", [128, 2, 128], F8),
    ("w2B", [128, 2, 128], F8),
    ("w3A", [128, 2, 128], F8),
    ("w3B", [128, 2, 128], F8),
    ("w4A", [128, 2, 128], F8),
    ("w4B", [128, 2, 128], F8),
    ("w4C", [128, 2, 128], F8),
    ("w4c", [1, 128], BF16),
    ("msk4", [1, 512], BF16),
    ("wfcl", [128, 16, 10], BF16),
    ("bias2d", [10, 512], F32),
    ("thr1n", [128, 1], F32),
    ("thr1t", [128, 1], F32),
    ("thr2n", [128, 1], F32),
    ("thr3n", [128, 1], F32),
    ("thr3t", [128, 1], F32),
    ("clB", [128, 1], F32),
    ("clA", [128, 1], F32),
]


# ---------------------------------------------------------------------------
# device program
# ---------------------------------------------------------------------------
def build_program(n_core=N_CORE, num_devices=N_CORES, debug_dump=False):
    nc = bacc.Bacc("TRN2", num_devices=num_devices)
    chunks = n_core // B
    x9_d = nc.dram_tensor(
        "x9", [chunks, 20, 32 * 6 * 68], F8, kind="ExternalInput"
    ).ap()
    P = {
        name: nc.dram_tensor(name, shape, dt, kind="ExternalInput").ap()
        for name, shape, dt in PARAM_SPECS
    }
    out = nc.dram_tensor("out", [10, n_core], F32, kind="ExternalOutput").ap()
    dump = None
    if debug_dump:
        dump = {
            "h3dump": nc.dram_tensor(
                "h3dump", [128, 128, 6, 16], F32, kind="ExternalOutput"
            ).ap(),
            "h4dump": nc.dram_tensor(
                "h4dump", [128, 128, 16], F32, kind="ExternalOutput"
            ).ap(),
        }
    with tile.TileContext(nc) as tc:
        _emit(nc, tc, x9_d, P, out, chunks, dump=dump)
    nc.compile()
    return nc


def _emit(nc, tc, x9_d, P, out, chunks, dump=None):
    from contextlib import ExitStack

    ctx = ExitStack()
    FB = min(4, chunks)  # chunks per FC block
    singles = ctx.enter_context(tc.tile_pool(name="singles", bufs=1))
    psA = ctx.enter_context(tc.tile_pool(name="psA", bufs=3, space="PSUM"))
    psB = ctx.enter_context(tc.tile_pool(name="psB", bufs=2, space="PSUM"))
    tmp3p = ctx.enter_context(tc.tile_pool(name="tmp3p", bufs=4))
    osbp = ctx.enter_context(tc.tile_pool(name="osbp", bufs=2))

    sb = {}
    for name, shape, dt in PARAM_SPECS:
        sb[name] = singles.tile(shape, dt, name=f"{name}_sb")
        nc.sync.dma_start(out=sb[name], in_=P[name])

    # persistent double-buffers
    x9b = [singles.tile([20, 32, 6 * 68], F8, name=f"x9_{i}") for i in range(2)]
    tmp1b = [singles.tile([128, 32, 6, 32], F8, name=f"tmp1_{i}") for i in range(2)]
    h1bb = [singles.tile([128, 34, 256], F8, name=f"h1b_{i}") for i in range(2)]
    q2b = [singles.tile([128, 66, 256], F8, name=f"q2_{i}") for i in range(2)]
    h3bb = [singles.tile([128, 128, 6, 16], F8, name=f"h3b_{i}") for i in range(2)]
    h4b = [
        singles.tile([128, FB * 128, 16], BF16, name=f"h4_{i}") for i in range(2)
    ]
    # init only halo/pad bytes (thresholds never write them); full-slot
    # memsets for the two border pad slots
    for i in range(2):
        for t, v, S in ((h1bb[i], 0.5, 34), (q2b[i], 0.0, 66)):
            base = t[:, 0, 0:1]
            ns = S - 2  # real slots 1..S-2
            nc.vector.memset(
                subap(base, [[256, ns], [34, 6], [1, 2]], extra=256 + 33), v
            )
            nc.vector.memset(subap(base, [[256, ns], [1, 1]], extra=256), v)
            nc.gpsimd.memset(
                subap(base, [[256, ns], [1, 51]], extra=256 + 205), v
            )
            nc.gpsimd.memset(t[:, 0], v)
            nc.gpsimd.memset(t[:, S - 1], v)

    def emit_dma(ci):
        nc.sync.dma_start(out=x9b[ci % 2], in_=x9_d[ci])

    def conv1_tile(ci, gp):
        """2 DR matmuls; ACT signs even-w into tmp1, DVE STT fuses the odd-w
        threshold with the pool-max into h1b ({0,1})."""
        x9t, tmp1 = x9b[ci % 2], tmp1b[ci % 2]
        h1bt = h1bb[ci % 2]
        pp = psA.tile([128, 2, 512], F32, tag="pp")
        for k in range(2):
            g = 2 * gp + k
            rhs = subap(x9t[:, g, 0:1], [[2, 2], [68, 6], [1, 64]])
            nc.tensor.matmul(
                pp[:, k, 0:384],
                lhsT=sb["w1dr"],
                rhs=rhs,
                start=True,
                stop=True,
                perf_mode=DR,
                tile_position=(0, 0),
            )
        ppv = pp[:, :, 0:384].rearrange("p a (h w) -> p a h w", h=6)
        te = tmp1[:, 2 * gp : 2 * gp + 2]
        nc.scalar.activation(te, ppv[:, :, :, 0:64:2], ACTF.Sign, bias=sb["thr1n"])
        for k in range(2):
            o = subap(
                h1bt[:, 0, 0:1],
                [[34, 6], [1, 32]],
                extra=(1 + 2 * gp + k) * 256 + 1,
            )
            nc.vector.scalar_tensor_tensor(
                o, ppv[:, k, :, 1:64:2], sb["thr1t"], te[:, k], ALU.is_ge, ALU.max
            )

    def conv2_tile(ci, k16):
        """2 slot-pairs (one 2-bank psum tile) + ACT Sign -> q2 (+-1)."""
        Pcp, jp = k16 // 8, k16 % 8
        h1bt, q2t = h1bb[ci % 2], q2b[ci % 2]
        pp = psA.tile([128, 2, 512], F32, tag="pp")
        for b2 in range(2):
            u0 = 1 + 4 * jp + 2 * b2
            for st, wname, d in ((True, "w2A", -1), (False, "w2B", 0)):
                rhs = subap(
                    h1bt[64 * Pcp : 64 * Pcp + 64, 0, 0:1],
                    [[2, 2], [1, 512]],
                    extra=u0 * 256 + d,
                )
                nc.tensor.matmul(
                    pp[:, b2, :],
                    lhsT=sb[wname][64 * Pcp : 64 * Pcp + 64],
                    rhs=rhs,
                    start=st,
                    stop=not st,
                    perf_mode=DR,
                    tile_position=(64 * Pcp, 0),
                )
        inp = subap(pp[:, 0, 0:1], [[512, 2], [256, 2], [34, 6], [1, 32]], extra=1)
        o = subap(
            q2t[:, 0, 0:1],
            [[512, 2], [256, 2], [34, 6], [1, 32]],
            extra=(1 + 32 * Pcp + 4 * jp) * 256 + 1,
        )
        nc.scalar.activation(o, inp, ACTF.Sign, bias=sb["thr2n"])

    def conv3_group(ci, gq, ta, tb):
        """2 psum tiles (t16 = ta, tb), thresholds, one pool op -> h3b.
        t16 < T16_SPLIT: ACT Sign (+-1); else DVE is_ge*2 ({0,2})."""
        q2t, h3bt = q2b[ci % 2], h3bb[ci % 2]
        t3 = tmp3p.tile([128, 8, 192], F8, tag="t3")
        for ti, t16 in enumerate((ta, tb)):
            pp = psA.tile([128, 2, 512], F32, tag="pp")
            for b2 in range(2):
                u0 = 1 + 4 * t16 + 2 * b2
                for st, wname, d in ((True, "w3A", -1), (False, "w3B", 0)):
                    rhs = subap(
                        q2t[64 * gq : 64 * gq + 64, 0, 0:1],
                        [[2, 2], [1, 512]],
                        extra=u0 * 256 + d,
                    )
                    nc.tensor.matmul(
                        pp[:, b2, :],
                        lhsT=sb[wname][64 * gq : 64 * gq + 64],
                        rhs=rhs,
                        start=st,
                        stop=not st,
                        perf_mode=DR,
                        tile_position=(64 * gq, 0),
                    )
            inp = subap(
                pp[:, 0, 0:1], [[512, 2], [256, 2], [34, 6], [1, 32]], extra=1
            )
            o = t3[:, 4 * ti : 4 * ti + 4, :].rearrange(
                "p s (h w) -> p s h w", h=6
            )
            if t16 < T16_SPLIT:
                nc.scalar.activation(o, inp, ACTF.Sign, bias=sb["thr3n"])
            else:
                nc.vector.tensor_scalar(
                    o, inp, sb["thr3t"], 2.0, ALU.is_ge, ALU.mult
                )
        sig0 = 64 * gq + 4 * ta
        o = subap(
            h3bt[:, 0, 0, 0:1],
            [[(tb - ta) * 384, 2], [96, 4], [16, 6], [1, 16]],
            extra=sig0 * 96,
        )
        in0 = subap(t3[:, 0, 0:1], [[768, 2], [192, 4], [32, 6], [2, 16]], extra=0)
        in1 = subap(t3[:, 0, 0:1], [[768, 2], [192, 4], [32, 6], [2, 16]], extra=1)
        nc.vector.tensor_tensor(o, in0, in1, ALU.max)

    # conv4 rounds r4 cover h3b slots [32*r4, 32*r4+32); within rounds 1 and
    # 3 the slots past 4*T16_SPLIT-32 are {0,2}-coded -> masked -S4 ones-row.

    def conv4_round(ci, r4):
        h3bt, h4t = h3bb[ci % 2], h4b[(ci // FB) % 2]
        pp4 = psB.tile([128, 512], F32, tag="pp4")
        ones = r4 % 2 == 1  # rounds 1 and 3 contain the {0,2}-coded slots
        for hp, wname in enumerate(("w4A", "w4B", "w4C")):
            rhs = subap(
                h3bt[:, 0, 0, 0:1],
                [[16, 2], [96, 32], [1, 16]],
                extra=32 * r4 * 96 + 2 * hp * 16,
            )
            nc.tensor.matmul(
                pp4,
                lhsT=sb[wname],
                rhs=rhs,
                start=(hp == 0),
                stop=(hp == 2 and not ones),
                perf_mode=DR,
                tile_position=(0, 0),
            )
        if ones:
            nc.tensor.matmul(
                pp4,
                lhsT=sb["w4c"],
                rhs=sb["msk4"],
                start=False,
                stop=True,
                tile_position=(0, 0),
            )
        nc.vector.tensor_scalar(
            h4t[
                :, 128 * (ci % FB) + 32 * r4 : 128 * (ci % FB) + 32 * r4 + 32, :
            ].rearrange("p n w -> p (n w)"),
            pp4,
            sb["clB"],
            sb["clA"],
            ALU.min,
            ALU.max,
        )

    def emit_fc(ci):
        ND = FB * 128
        h4t = h4b[(ci // FB) % 2]
        ppf = psB.tile([128, 512], F32, tag="pp4")
        for w in range(16):
            nc.tensor.matmul(
                ppf[0:10, 0:ND],
                lhsT=sb["wfcl"][:, w, :],
                rhs=h4t[:, :, w],
                start=(w == 0),
                stop=(w == 15),
            )
        osb = osbp.tile([16, 512], F32, tag="osb")
        nc.vector.tensor_tensor(
            osb[0:10, 0:ND], ppf[0:10, 0:ND], sb["bias2d"][:, 0:ND], ALU.add
        )
        blk = ci // FB
        nc.sync.dma_start(
            out=out[:, blk * ND : (blk + 1) * ND], in_=osb[0:10, 0:ND]
        )

    # ---- emission: software-pipelined across chunks -----------------------
    # conv3 groups pair an ACT-coded tile with a DVE-coded one where possible
    PAIRS3 = [(0, 10), (1, 11), (2, 12), (3, 13), (4, 14), (5, 15), (6, 7), (8, 9)]
    ORD3 = [(gq, ta, tb) for ta, tb in PAIRS3 for gq in range(2)]
    emit_dma(0)
    for gp in range(16):
        conv1_tile(0, gp)
    if chunks > 1:
        emit_dma(1)
    for k16 in range(16):
        conv2_tile(0, k16)
        if chunks > 1:
            conv1_tile(1, k16)
    for ci in range(chunks):
        if ci + 2 < chunks:
            emit_dma(ci + 2)
        for k16 in range(16):
            gq, ta, tb = ORD3[k16]
            conv3_group(ci, gq, ta, tb)
            if ci + 1 < chunks:
                conv2_tile(ci + 1, k16)
            if ci + 2 < chunks:
                conv1_tile(ci + 2, k16)
        for r4 in range(4):
            conv4_round(ci, r4)
        if ci % FB == FB - 1:
            emit_fc(ci)
    if dump is not None:
        h3f = singles.tile([128, 128, 6, 16], F32, name="h3f_dbg")
        nc.vector.tensor_copy(h3f, h3bb[0])
        nc.sync.dma_start(out=dump["h3dump"], in_=h3f)
        h4f = singles.tile([128, 128, 16], F32, name="h4f_dbg")
        nc.vector.tensor_copy(h4f, h4b[0][:, 0:128, :])
        nc.sync.dma_start(out=dump["h4dump"], in_=h4f)
    ctx.close()


# ---------------------------------------------------------------------------
# entry point
# ---------------------------------------------------------------------------
_PROGRAM = None


def kernel(**inputs):
    from concourse.bass_utils import run_bass_kernel_spmd

    global _PROGRAM
    if _PROGRAM is None:
        _PROGRAM = build_program()
    nc = _PROGRAM

    params = host_prep_params(inputs)
    x = np.ascontiguousarray(inputs["x"], dtype=np.float32).reshape(N_TOTAL, 6, 128)
    x9 = host_pack_x(x)  # (64, 20, 13056)
    chunks = N_CORE // B
    in_maps = []
    for i in range(N_CORES):
        m = dict(params)
        m["x9"] = x9[i * chunks : (i + 1) * chunks]
        in_maps.append(m)
    res = run_bass_kernel_spmd(nc, in_maps, core_ids=list(range(N_CORES)))
    outs = [res.results[i]["out"].T for i in range(N_CORES)]  # (1024, 10) each
    return np.ascontiguousarray(np.concatenate(outs, axis=0)).astype(np.float32)


# revision 4
# speedup vs baseline: 2966.3058x; 1.0357x over previous
"""Binary CNN Trainium2 kernel — 8-core data parallel, DoubleRow-centric.

Host packs sign(x) into stride-2 tap-pair rows so conv1 is one DoubleRow
matmul per 4-sample group (K=40 block-diagonal, M=128) with zero on-device
input prep and no DRAM staging. conv2/conv3 use a flat (6h x 34w + pad)
halo'd slot layout ([slot, 256] fp8) so each tile is exactly 2 stride-2
DoubleRow matmuls (taps (0,2) then (1,zero)) at FD=512; conv4 is 3 DoubleRow
matmuls over h-pairs. Class pairs / channel halves sit at different
partition bases so their matmuls share the PE array on real HW.

Elementwise work is balanced across ACT and DVE: conv1's boundary fuses the
odd-w threshold with the pool-max via scalar_tensor_tensor (DVE) after an
ACT Sign on the even half; conv2's threshold runs on ACT (+-1 q2); conv3's
threshold splits ACT (Sign, +-1) / DVE (is_ge*2, {0,2}) by sample range with
a ones-row psum correction in conv4, then a DVE tensor_tensor max pools into
h3b. BN4+hardtanh+FC fold into a per-channel clamp plus host-folded FC
weights; FC is weight-stationary over 4-chunk blocks producing [10, N] that
the host transposes.
"""

import numpy as np
import ml_dtypes

import concourse.bass as bass
import concourse.bacc as bacc
import concourse.tile as tile
import concourse.mybir as mybir

F32 = mybir.dt.float32
BF16 = mybir.dt.bfloat16
F8 = mybir.dt.float8e4
NPF8 = ml_dtypes.float8_e4m3
NPBF16 = ml_dtypes.bfloat16
BN_EPS = 1e-5

N_CORES = 8
N_TOTAL = 8192
N_CORE = N_TOTAL // N_CORES  # 1024
B = 128  # samples per chunk
ALU = mybir.AluOpType
ACTF = mybir.ActivationFunctionType
DR = mybir.MatmulPerfMode.DoubleRow
T16_SPLIT = 10  # conv3 tiles t16 < split: ACT (+-1); rest: DVE ({0,2})

# conv1 tap-pair rows: (base shift, parity); sub0 -> shift base, sub1 -> base+2
ROWS1 = [(-2, 0), (-1, 0), (2, 0), (-2, 1), (-1, 1)]
# taps covered: (r, sub) -> original tap index t (None = zero weight)
TAP1 = [(0, 4), (2, 6), (8, None), (1, 5), (3, 7)]


def subap(sl, dims, extra=0):
    return bass.AP(
        tensor=sl.tensor, offset=sl.offset + extra, ap=[list(sl.ap[0])] + dims
    )


# ---------------------------------------------------------------------------
# host-side preparation
# ---------------------------------------------------------------------------
def host_prep_params(p):
    def s(k):
        return (p[f"g{k}"] / np.sqrt(p[f"v{k}"] + BN_EPS)).astype(np.float32)

    w1b = np.sign(p["w1"]).astype(np.float32)[:, 0, 0, :]  # (32, 9)
    w2b = np.sign(p["w2"]).astype(np.float32)[:, :, 0, :]  # (64, 32, 3)
    w3b = np.sign(p["w3"]).astype(np.float32)[:, :, 0, :]  # (128, 64, 3)
    w4b = np.sign(p["w4"]).astype(np.float32)[:, :, :, 0]  # (128, 128, 6)
    s1, s2, s3, s4 = s(1), s(2), s(3), s(4)
    thr1 = (p["m1"] - p["b1"] - p["be1"] / s1).astype(np.float32)  # (32,)
    thr2 = (p["m2"] - p["b2"] - p["be2"] / s2).astype(np.float32)  # (64,)
    S2 = w2b.sum(axis=(1, 2))
    thr2_01 = ((thr2 + S2) / 2).astype(np.float32)  # h1b {0,1} coded
    thr3 = (p["m3"] - p["be3"] / s3 - p["b3"]).astype(np.float32)  # q2 +-1 coded
    S4 = w4b.sum(axis=(1, 2)).astype(np.float32)  # {0,2} coding correction

    # conv1 lhsT [20, 2, 128]
    w1dr = np.zeros((20, 2, 128), NPF8)
    for si in range(4):
        for r in range(5):
            for k in range(2):
                t = TAP1[r][k]
                if t is not None:
                    w1dr[5 * si + r, k, 32 * si : 32 * si + 32] = w1b[:, t].astype(
                        NPF8
                    )

    # conv2 lhsT [128, 2, 128]: rows (32*ci2 + c), cols (64*ci2 + d); dup halves
    w2A = np.zeros((128, 2, 128), NPF8)
    w2B = np.zeros((128, 2, 128), NPF8)
    for ci2 in range(2):
        w2A[32 * ci2 : 32 * ci2 + 32, 0, 64 * ci2 : 64 * ci2 + 64] = w2b[
            :, :, 0
        ].T.astype(NPF8)
        w2A[32 * ci2 : 32 * ci2 + 32, 1, 64 * ci2 : 64 * ci2 + 64] = w2b[
            :, :, 2
        ].T.astype(NPF8)
        w2B[32 * ci2 : 32 * ci2 + 32, 0, 64 * ci2 : 64 * ci2 + 64] = w2b[
            :, :, 1
        ].T.astype(NPF8)
    w2A[64:128] = w2A[0:64]
    w2B[64:128] = w2B[0:64]

    # conv3 lhsT [128, 2, 128]: rows c (64), cols e (128); dup halves
    w3A = np.zeros((128, 2, 128), NPF8)
    w3B = np.zeros((128, 2, 128), NPF8)
    w3A[0:64, 0, :] = w3b[:, :, 0].T.astype(NPF8)
    w3A[0:64, 1, :] = w3b[:, :, 2].T.astype(NPF8)
    w3B[0:64, 0, :] = w3b[:, :, 1].T.astype(NPF8)
    w3A[64:128] = w3A[0:64]
    w3B[64:128] = w3B[0:64]

    # conv4 DoubleRow lhsT over h-pairs: 3 tiles [128, 2, 128]: [c, k, o]
    w4A = np.ascontiguousarray(w4b[:, :, 0:2].transpose(1, 2, 0)).astype(NPF8)
    w4B = np.ascontiguousarray(w4b[:, :, 2:4].transpose(1, 2, 0)).astype(NPF8)
    w4C = np.ascontiguousarray(w4b[:, :, 4:6].transpose(1, 2, 0)).astype(NPF8)

    # BN4 + hardtanh + FC folds (h3b +-1 coded; {0,2} ranges get a -S4
    # ones-row inside psum so one clamp/bias set serves all samples).
    b4 = p["b4"].astype(np.float32)
    m4 = p["m4"].astype(np.float32)
    be4 = p["be4"].astype(np.float32)
    A4 = (-b4 + m4 - (1.0 + be4) / s4).astype(np.float32)
    B4 = (-b4 + m4 + (1.0 - be4) / s4).astype(np.float32)
    D4 = (s4 * (b4 - m4) + be4).astype(np.float32)
    wfc = p["wfc"].astype(np.float32)  # (10, 2048), k = c*16 + w
    wfc2 = wfc * np.repeat(s4, 16)[None, :]
    bfc2 = (p["bfc"].astype(np.float32) + wfc @ np.repeat(D4, 16)).astype(
        np.float32
    )
    wfcl = np.zeros((128, 16, 10), NPBF16)
    for w in range(16):
        wfcl[:, w, :] = wfc2[:, w::16].T.astype(NPBF16)  # [c, j]
    bias2d = np.ascontiguousarray(
        np.broadcast_to(bfc2[:, None], (10, 512))
    ).astype(np.float32)

    def col(v):
        return np.ascontiguousarray(v.reshape(-1, 1)).astype(np.float32)

    return {
        "w1dr": w1dr,
        "w2A": w2A,
        "w2B": w2B,
        "w3A": w3A,
        "w3B": w3B,
        "w4A": w4A,
        "w4B": w4B,
        "w4C": w4C,
        "w4c": np.ascontiguousarray((-S4).reshape(1, 128)).astype(NPBF16),
        "msk4": np.ascontiguousarray(
            np.repeat((np.arange(32) >= 8).astype(np.float32), 16).reshape(1, 512)
        ).astype(NPBF16),
        "wfcl": wfcl,
        "bias2d": bias2d,
        "thr1n": col(np.tile(-thr1, 4)),
        "thr1t": col(np.tile(thr1, 4)),
        "thr2n": col(np.tile(-thr2_01, 2)),
        "thr3n": col(-thr3),
        "thr3t": col(thr3),
        "clB": col(B4),
        "clA": col(A4),
    }


_NIDX = None  # per-chunk sample permutation n(s, g) = 64*(s%2) + 32*(s//2) + g


def _nidx():
    global _NIDX
    if _NIDX is None:
        idx = np.zeros((4, 32), np.int64)
        for si in range(4):
            idx[si] = 64 * (si % 2) + 32 * (si // 2) + np.arange(32)
        _NIDX = idx
    return _NIDX


def host_pack_x(x):
    """x (N, 6, 128) f32 -> x9 (N//128, 20, 32*6*68) f8."""
    N = x.shape[0]
    xb = np.sign(x).astype(NPF8)
    par = xb.reshape(N, 6, 64, 2)  # [..., w2, parity]
    rows = np.zeros((N, 5, 6, 68), NPF8)
    for r, (base, parity) in enumerate(ROWS1):
        src = par[..., parity]  # (N, 6, 64)
        lo = max(0, -base)
        hi = min(66, 64 - base)
        rows[:, r, :, lo:hi] = src[:, :, base + lo : base + hi]
    chunks = N // B
    rows = rows.reshape(chunks, B, 5, 6 * 68)
    x9 = np.empty((chunks, 4, 32, 5, 6 * 68), NPF8)
    idx = _nidx()
    for si in range(4):
        x9[:, si] = rows[:, idx[si]]
    x9 = np.ascontiguousarray(x9.transpose(0, 1, 3, 2, 4))
    return x9.reshape(chunks, 20, 32 * 6 * 68)


PARAM_SPECS = [
    ("w1dr", [20, 2, 128], F8),
    ("w2A", [128, 2, 128], F8),
    ("w2B", [128, 2, 128], F8),
    ("w3A", [128, 2, 128], F8),
    ("w3B", [128, 2, 128], F8),
    ("w4A", [128, 2, 128], F8),
    ("w4B", [128, 2, 128], F8),
    ("w4C", [128, 2, 128], F8),
    ("w4c", [1, 128], BF16),
    ("msk4", [1, 512], BF16),
    ("wfcl", [128, 16, 10], BF16),
    ("bias2d", [10, 512], F32),
    ("thr1n", [128, 1], F32),
    ("thr1t", [128, 1], F32),
    ("thr2n", [128, 1], F32),
    ("thr3n", [128, 1], F32),
    ("thr3t", [128, 1], F32),
    ("clB", [128, 1], F32),
    ("clA", [128, 1], F32),
]


# ---------------------------------------------------------------------------
# device program
# ---------------------------------------------------------------------------
def build_program(n_core=N_CORE, num_devices=N_CORES, debug_dump=False):
    nc = bacc.Bacc("TRN2", num_devices=num_devices)
    chunks = n_core // B
    x9_d = nc.dram_tensor(
        "x9", [chunks, 20, 32 * 6 * 68], F8, kind="ExternalInput"
    ).ap()
    P = {
        name: nc.dram_tensor(name, shape, dt, kind="ExternalInput").ap()
        for name, shape, dt in PARAM_SPECS
    }
    out = nc.dram_tensor("out", [10, n_core], F32, kind="ExternalOutput").ap()
    dump = None
    if debug_dump:
        dump = {
            "h3dump": nc.dram_tensor(
                "h3dump", [128, 128, 6, 16], F32, kind="ExternalOutput"
            ).ap(),
            "h4dump": nc.dram_tensor(
                "h4dump", [128, 128, 16], F32, kind="ExternalOutput"
            ).ap(),
        }
    with tile.TileContext(nc) as tc:
        _emit(nc, tc, x9_d, P, out, chunks, dump=dump)
    nc.compile()
    return nc


def _emit(nc, tc, x9_d, P, out, chunks, dump=None):
    from contextlib import ExitStack

    ctx = ExitStack()
    FB = min(4, chunks)  # chunks per FC block
    singles = ctx.enter_context(tc.tile_pool(name="singles", bufs=1))
    psA = ctx.enter_context(tc.tile_pool(name="psA", bufs=3, space="PSUM"))
    psB = ctx.enter_context(tc.tile_pool(name="psB", bufs=2, space="PSUM"))
    tmp3p = ctx.enter_context(tc.tile_pool(name="tmp3p", bufs=4))
    osbp = ctx.enter_context(tc.tile_pool(name="osbp", bufs=2))

    sb = {}
    for name, shape, dt in PARAM_SPECS:
        sb[name] = singles.tile(shape, dt, name=f"{name}_sb")
        nc.sync.dma_start(out=sb[name], in_=P[name])

    # persistent double-buffers
    x9b = [singles.tile([20, 32, 6 * 68], F8, name=f"x9_{i}") for i in range(2)]
    tmp1b = [singles.tile([128, 32, 6, 32], F8, name=f"tmp1_{i}") for i in range(2)]
    h1bb = [singles.tile([128, 34, 256], F8, name=f"h1b_{i}") for i in range(2)]
    q2b = [singles.tile([128, 66, 256], F8, name=f"q2_{i}") for i in range(2)]
    h3bb = [singles.tile([128, 128, 6, 16], F8, name=f"h3b_{i}") for i in range(2)]
    h4b = [
        singles.tile([128, FB * 128, 16], BF16, name=f"h4_{i}") for i in range(2)
    ]
    # init only halo/pad bytes (thresholds never write them); full-slot
    # memsets for the two border pad slots
    for i in range(2):
        for t, v, S in ((h1bb[i], 0.5, 34), (q2b[i], 0.0, 66)):
            base = t[:, 0, 0:1]
            ns = S - 2  # real slots 1..S-2
            nc.vector.memset(
                subap(base, [[256, ns], [34, 6], [1, 2]], extra=256 + 33), v
            )
            nc.vector.memset(subap(base, [[256, ns], [1, 1]], extra=256), v)
            nc.gpsimd.memset(
                subap(base, [[256, ns], [1, 51]], extra=256 + 205), v
            )
            nc.gpsimd.memset(t[:, 0], v)
            nc.gpsimd.memset(t[:, S - 1], v)

    def emit_dma(ci):
        nc.sync.dma_start(out=x9b[ci % 2], in_=x9_d[ci])

    def conv1_tile(ci, gp):
        """2 DR matmuls; ACT signs even-w into tmp1, DVE STT fuses the odd-w
        threshold with the pool-max into h1b ({0,1})."""
        x9t, tmp1 = x9b[ci % 2], tmp1b[ci % 2]
        h1bt = h1bb[ci % 2]
        pp = psA.tile([128, 2, 512], F32, tag="pp")
        for k in range(2):
            g = 2 * gp + k
            rhs = subap(x9t[:, g, 0:1], [[2, 2], [68, 6], [1, 64]])
            nc.tensor.matmul(
                pp[:, k, 0:384],
                lhsT=sb["w1dr"],
                rhs=rhs,
                start=True,
                stop=True,
                perf_mode=DR,
                tile_position=(0, 0),
            )
        ppv = pp[:, :, 0:384].rearrange("p a (h w) -> p a h w", h=6)
        te = tmp1[:, 2 * gp : 2 * gp + 2]
        nc.scalar.activation(te, ppv[:, :, :, 0:64:2], ACTF.Sign, bias=sb["thr1n"])
        for k in range(2):
            o = subap(
                h1bt[:, 0, 0:1],
                [[34, 6], [1, 32]],
                extra=(1 + 2 * gp + k) * 256 + 1,
            )
            nc.vector.scalar_tensor_tensor(
                o, ppv[:, k, :, 1:64:2], sb["thr1t"], te[:, k], ALU.is_ge, ALU.max
            )

    def conv2_tile(ci, k16):
        """2 slot-pairs (one 2-bank psum tile) + ACT Sign -> q2 (+-1)."""
        Pcp, jp = k16 // 8, k16 % 8
        h1bt, q2t = h1bb[ci % 2], q2b[ci % 2]
        pp = psA.tile([128, 2, 512], F32, tag="pp")
        for b2 in range(2):
            u0 = 1 + 4 * jp + 2 * b2
            for st, wname, d in ((True, "w2A", -1), (False, "w2B", 0)):
                rhs = subap(
                    h1bt[64 * Pcp : 64 * Pcp + 64, 0, 0:1],
                    [[2, 2], [1, 512]],
                    extra=u0 * 256 + d,
                )
                nc.tensor.matmul(
                    pp[:, b2, :],
                    lhsT=sb[wname][64 * Pcp : 64 * Pcp + 64],
                    rhs=rhs,
                    start=st,
                    stop=not st,
                    perf_mode=DR,
                    tile_position=(64 * Pcp, 0),
                )
        inp = subap(pp[:, 0, 0:1], [[512, 2], [256, 2], [34, 6], [1, 32]], extra=1)
        o = subap(
            q2t[:, 0, 0:1],
            [[512, 2], [256, 2], [34, 6], [1, 32]],
            extra=(1 + 32 * Pcp + 4 * jp) * 256 + 1,
        )
        nc.scalar.activation(o, inp, ACTF.Sign, bias=sb["thr2n"])

    def conv3_group(ci, gq, ta, tb, mid=None):
        """2 psum tiles (t16 = ta, tb), thresholds, one pool op -> h3b.
        t16 < T16_SPLIT: ACT Sign (+-1); else DVE is_ge*2 ({0,2})."""
        q2t, h3bt = q2b[ci % 2], h3bb[ci % 2]
        t3 = tmp3p.tile([128, 8, 192], F8, tag="t3")
        for ti, t16 in enumerate((ta, tb)):
            if ti == 1 and mid is not None:
                mid()
            pp = psA.tile([128, 2, 512], F32, tag="pp")
            for b2 in range(2):
                u0 = 1 + 4 * t16 + 2 * b2
                for st, wname, d in ((True, "w3A", -1), (False, "w3B", 0)):
                    rhs = subap(
                        q2t[64 * gq : 64 * gq + 64, 0, 0:1],
                        [[2, 2], [1, 512]],
                        extra=u0 * 256 + d,
                    )
                    nc.tensor.matmul(
                        pp[:, b2, :],
                        lhsT=sb[wname][64 * gq : 64 * gq + 64],
                        rhs=rhs,
                        start=st,
                        stop=not st,
                        perf_mode=DR,
                        tile_position=(64 * gq, 0),
                    )
            inp = subap(
                pp[:, 0, 0:1], [[512, 2], [256, 2], [34, 6], [1, 32]], extra=1
            )
            o = t3[:, 4 * ti : 4 * ti + 4, :].rearrange(
                "p s (h w) -> p s h w", h=6
            )
            if t16 < T16_SPLIT:
                nc.scalar.activation(o, inp, ACTF.Sign, bias=sb["thr3n"])
            else:
                nc.vector.tensor_scalar(
                    o, inp, sb["thr3t"], 2.0, ALU.is_ge, ALU.mult
                )
        sig0 = 64 * gq + 4 * ta
        o = subap(
            h3bt[:, 0, 0, 0:1],
            [[(tb - ta) * 384, 2], [96, 4], [16, 6], [1, 16]],
            extra=sig0 * 96,
        )
        in0 = subap(t3[:, 0, 0:1], [[768, 2], [192, 4], [32, 6], [2, 16]], extra=0)
        in1 = subap(t3[:, 0, 0:1], [[768, 2], [192, 4], [32, 6], [2, 16]], extra=1)
        nc.vector.tensor_tensor(o, in0, in1, ALU.max)

    # conv4 rounds r4 cover h3b slots [32*r4, 32*r4+32); within rounds 1 and
    # 3 the slots past 4*T16_SPLIT-32 are {0,2}-coded -> masked -S4 ones-row.

    def conv4_round(ci, r4):
        h3bt, h4t = h3bb[ci % 2], h4b[(ci // FB) % 2]
        pp4 = psB.tile([128, 512], F32, tag="pp4")
        ones = r4 % 2 == 1  # rounds 1 and 3 contain the {0,2}-coded slots
        for hp, wname in enumerate(("w4A", "w4B", "w4C")):
            rhs = subap(
                h3bt[:, 0, 0, 0:1],
                [[16, 2], [96, 32], [1, 16]],
                extra=32 * r4 * 96 + 2 * hp * 16,
            )
            nc.tensor.matmul(
                pp4,
                lhsT=sb[wname],
                rhs=rhs,
                start=(hp == 0),
                stop=(hp == 2 and not ones),
                perf_mode=DR,
                tile_position=(0, 0),
            )
        if ones:
            nc.tensor.matmul(
                pp4,
                lhsT=sb["w4c"],
                rhs=sb["msk4"],
                start=False,
                stop=True,
                tile_position=(0, 0),
            )
        nc.vector.tensor_scalar(
            h4t[
                :, 128 * (ci % FB) + 32 * r4 : 128 * (ci % FB) + 32 * r4 + 32, :
            ].rearrange("p n w -> p (n w)"),
            pp4,
            sb["clB"],
            sb["clA"],
            ALU.min,
            ALU.max,
        )

    def emit_fc(ci):
        ND = FB * 128
        h4t = h4b[(ci // FB) % 2]
        ppf = psB.tile([128, 512], F32, tag="pp4")
        for w in range(16):
            nc.tensor.matmul(
                ppf[0:10, 0:ND],
                lhsT=sb["wfcl"][:, w, :],
                rhs=h4t[:, :, w],
                start=(w == 0),
                stop=(w == 15),
            )
        osb = osbp.tile([16, 512], F32, tag="osb")
        nc.vector.tensor_tensor(
            osb[0:10, 0:ND], ppf[0:10, 0:ND], sb["bias2d"][:, 0:ND], ALU.add
        )
        blk = ci // FB
        nc.sync.dma_start(
            out=out[:, blk * ND : (blk + 1) * ND], in_=osb[0:10, 0:ND]
        )

    # ---- emission: software-pipelined across chunks -----------------------
    # conv3 groups pair an ACT-coded tile with a DVE-coded one where possible
    PAIRS3 = [(0, 10), (1, 11), (2, 12), (3, 13), (4, 14), (5, 15), (6, 7), (8, 9)]
    ORD3 = [(gq, ta, tb) for ta, tb in PAIRS3 for gq in range(2)]
    emit_dma(0)
    for gp in range(16):
        conv1_tile(0, gp)
    if chunks > 1:
        emit_dma(1)
    for k16 in range(16):
        conv2_tile(0, k16)
        if chunks > 1:
            conv1_tile(1, k16)
    for ci in range(chunks):
        if ci + 2 < chunks:
            emit_dma(ci + 2)
        for k16 in range(16):
            gq, ta, tb = ORD3[k16]
            if ci + 1 < chunks:
                conv2_tile(ci + 1, k16)
            conv3_group(ci, gq, ta, tb)
            if ci + 2 < chunks:
                conv1_tile(ci + 2, k16)
        for r4 in range(4):
            conv4_round(ci, r4)
        if ci % FB == FB - 1:
            emit_fc(ci)
    if dump is not None:
        h3f = singles.tile([128, 128, 6, 16], F32, name="h3f_dbg")
        nc.vector.tensor_copy(h3f, h3bb[0])
        nc.sync.dma_start(out=dump["h3dump"], in_=h3f)
        h4f = singles.tile([128, 128, 16], F32, name="h4f_dbg")
        nc.vector.tensor_copy(h4f, h4b[0][:, 0:128, :])
        nc.sync.dma_start(out=dump["h4dump"], in_=h4f)
    ctx.close()


# ---------------------------------------------------------------------------
# entry point
# ---------------------------------------------------------------------------
_PROGRAM = None


def kernel(**inputs):
    from concourse.bass_utils import run_bass_kernel_spmd

    global _PROGRAM
    if _PROGRAM is None:
        _PROGRAM = build_program()
    nc = _PROGRAM

    params = host_prep_params(inputs)
    x = np.ascontiguousarray(inputs["x"], dtype=np.float32).reshape(N_TOTAL, 6, 128)
    x9 = host_pack_x(x)  # (64, 20, 13056)
    chunks = N_CORE // B
    in_maps = []
    for i in range(N_CORES):
        m = dict(params)
        m["x9"] = x9[i * chunks : (i + 1) * chunks]
        in_maps.append(m)
    res = run_bass_kernel_spmd(nc, in_maps, core_ids=list(range(N_CORES)))
    outs = [res.results[i]["out"].T for i in range(N_CORES)]  # (1024, 10) each
    return np.ascontiguousarray(np.concatenate(outs, axis=0)).astype(np.float32)
